# revision 43
# baseline (speedup 1.0000x reference)
"""Adaptive embedding lookup (nn.AdaptiveEmbedding) on 8 TRN2 NeuronCores.

Strategy (data-parallel over tokens, tables replicated, no collectives):

Host:
  - Bucket the 16384 tokens by embedding cluster (4 clusters; cluster 2 is
    further split into 5 sub-ranges of 32000 rows so dma_gather's int16
    indices stay in range, and cluster 3's table is packed 8-rows-per-256B-
    super-row), deal each bucket's tokens round-robin to the 8 cores, pad
    each per-core bucket to a multiple of 128 (one PE tile = one output
    "group" of 128 tokens).
  - Tables/projections pre-converted: emb0/emb1 bf16, emb2 f32 (256B rows
    for dma_gather), emb3 packed bf16 [, 128]; projections pre-transposed,
    pre-scaled by sqrt(d_proj), bf16, chunk-major for direct SBUF DMA.

Device (SPMD, identical graph on all 8 cores, one TileContext):
  - One dma_gather per bucket pulls this core's embedding rows into SBUF
    [128, n_groups, row_elems] (gathered row m lands at partition m%128,
    group m//128); indices are the int16 [128, n/16] wrapped+replicated
    layout the Q7 gather kernel expects.
  - Per 128-token group: (cluster 3: mask-select the 16-elem sub-row inside
    the 128-elem super-row; cluster 2: cast f32->bf16), PE-transpose the
    gathered rows, matmul against projection chunks accumulating in PSUM,
    copy (f32->bf16 cast) to SBUF, DMA the [128, 1024] output rows out.
    The Tile scheduler overlaps gathers, PE work, copies and output DMAs.

Host: inverse-permute the 8 per-core outputs into [8, 2048, 1024] f32.
"""

import numpy as np
import ml_dtypes

import concourse.bacc as bacc
import concourse.bass as bass
import concourse.mybir as mybir
from concourse.bass_utils import run_bass_kernel_spmd
from concourse.tile import TileContext

N_TOKEN = 267735
D_PROJ = 1024
CUTOFF_ENDS = [0, 20000, 40000, 200000, 267735]
D_EMBS = [1024, 256, 64, 16]
EMB_SCALE = float(D_PROJ) ** 0.5
N_CORES = 8
P = 128
NFREE = 512          # psum free-dim per matmul
C2_SUB = 32000       # cluster-2 subtable rows (int16 range)
C2_NSUB = 5
C3_PACK = 8          # cluster-3 rows packed per super-row
C3_SROWS = -(-(CUTOFF_ENDS[4] - CUTOFF_ENDS[3]) // C3_PACK)  # 8467

BF16 = ml_dtypes.bfloat16

# Test-harness knobs (the grader never touches these).
TRACE = False
TRACE_CORES = None
LAST = {}

_GRAPH_CACHE = {}

# unit = gather bucket: 0, 1, (2, r) for sub-range r, 3.
UNIT_KEYS = [0, 1] + [(2, r) for r in range(C2_NSUB)] + [3]


def _build_graph(Ks):
    """Ks: dict unit_key -> group count (0 allowed). Same on all cores."""
    key = tuple(Ks[u] for u in UNIT_KEYS)
    if key in _GRAPH_CACHE:
        return _GRAPH_CACHE[key]

    K3 = Ks[3]
    NI = 8 * sum(Ks.values())          # idx16 columns (8 per group)
    G = sum(Ks.values())               # total output groups
    NAUX = 128 + max(K3, 1) * 128      # [identity | c3 sub-row select mask]
    K2 = sum(Ks[(2, r)] for r in range(C2_NSUB))

    nc = bacc.Bacc("TRN2", debug=False, num_swdge_queues=4)
    idx_ext = nc.declare_dram_parameter("idx16", [P, max(NI, 16)], mybir.dt.int16, False)
    emb0_ext = nc.declare_dram_parameter("emb0b", [20000, 1024], mybir.dt.bfloat16, False)
    emb1_ext = nc.declare_dram_parameter("emb1b", [20000, 256], mybir.dt.bfloat16, False)
    emb2_ext = nc.declare_dram_parameter("emb2f", [160000, 64], mybir.dt.float32, False)
    emb3_ext = nc.declare_dram_parameter("emb3p", [C3_SROWS, 128], mybir.dt.bfloat16, False)
    pt_exts = [
        nc.declare_dram_parameter("pt0", [128, 8, D_PROJ], mybir.dt.bfloat16, False),
        nc.declare_dram_parameter("pt1", [128, 2, D_PROJ], mybir.dt.bfloat16, False),
        nc.declare_dram_parameter("pt2", [64, 1, D_PROJ], mybir.dt.bfloat16, False),
        nc.declare_dram_parameter("pt3s", [128, 1, D_PROJ], mybir.dt.bfloat16, False),
    ]
    aux_ext = nc.declare_dram_parameter("aux", [P, NAUX], mybir.dt.bfloat16, False)
    out_ext = nc.declare_dram_parameter("out", [G * P, D_PROJ], mybir.dt.bfloat16, True)

    with TileContext(nc) as tc:
        with tc.tile_pool(name="const", bufs=1) as constp, \
             tc.tile_pool(name="work", bufs=4) as workp, \
             tc.tile_pool(name="ps_o", bufs=4, space="PSUM") as psump, \
             tc.tile_pool(name="ps_t", bufs=2, space="PSUM") as psumtr, \
             tc.tile_pool(name="ps_w", bufs=1, space="PSUM") as psumw:
            idx_sb = constp.tile([P, max(NI, 16)], mybir.dt.int16, tag="idx")
            nc.sync.dma_start(out=idx_sb[:], in_=idx_ext[:])
            aux_sb = constp.tile([P, NAUX], mybir.dt.bfloat16, tag="aux")
            nc.sync.dma_start(out=aux_sb[:], in_=aux_ext[:])
            ident = aux_sb[:, 0:128]

            # c0/c1 are gathered pre-transposed; c0 is group-major so each
            # 128-token group can be gathered (and consumed) independently
            e0 = constp.tile([P, max(Ks[0], 1), 8, P], mybir.dt.bfloat16, tag="e0")
            e1 = constp.tile([P, 2, max(Ks[1], 1) * P], mybir.dt.bfloat16, tag="e1")
            e2 = constp.tile([P, max(K2, 1), 64], mybir.dt.float32, tag="e2")
            e3 = constp.tile([P, max(K3, 1), 128], mybir.dt.bfloat16, tag="e3")

            # idx16 column offset per unit (indices are packed in UNIT_KEYS order)
            unit_col = {}
            col = 0
            for u in UNIT_KEYS:
                unit_col[u] = col
                col += 8 * Ks[u]
            # group base within each cluster's e-seg, in UNIT_KEYS order
            unit_j0 = {}
            jb = {0: 0, 1: 0, 2: 0, 3: 0}
            for u in UNIT_KEYS:
                cbase = 2 if isinstance(u, tuple) else u
                unit_j0[u] = jb[cbase]
                jb[cbase] += Ks[u]

            # gather emission order: c0 split per group across queues 0/1 so
            # the first matmuls can start as soon as group 0 lands; c1/c3
            # early on the other queues; cluster-2 sub-gathers fill in
            gather_list = []  # (unit, group_lo, n_groups)
            for glo in range(Ks[0]):
                gather_list.append((0, glo, 1))
            for u in [1, 3, (2, 0), (2, 1), (2, 2), (2, 3), (2, 4)]:
                if Ks[u] > 0:
                    gather_list.append((u, 0, Ks[u]))
            for qi, (u, glo, n) in enumerate(gather_list):
                j0 = unit_j0[u] + glo
                tr_mode = u in (0, 1)
                if u == 0:
                    dst, tab, elem = e0[:, glo, :, :], emb0_ext[:], 1024
                elif u == 1:
                    dst, tab, elem = e1[:], emb1_ext[:], 256
                elif u == 3:
                    dst, tab, elem = e3[:, j0:j0 + n, :], emb3_ext[:], 128
                else:
                    r = u[1]
                    dst, tab, elem = (
                        e2[:, j0:j0 + n, :],
                        emb2_ext[r * C2_SUB:(r + 1) * C2_SUB, :],
                        64,
                    )
                c0 = unit_col[u] + 8 * glo
                nc.gpsimd.dma_gather(
                    dst, tab, idx_sb[:, c0:c0 + 8 * n], n * P, n * P, elem,
                    transpose=tr_mode,
                    queue_num=qi % 4,
                )

            pt_sb = []
            for i, (pc, ch) in enumerate(((128, 8), (128, 2), (64, 1), (128, 1))):
                t = constp.tile([pc, ch, D_PROJ], mybir.dt.bfloat16, tag=f"pt{i}")
                nc.sync.dma_start(out=t[:], in_=pt_exts[i][:])
                pt_sb.append(t)

            # PE warmup: a dense burst of throwaway transposes keeps the PE
            # clock ramping toward HAM while the gathers are still in flight
            wps = psumw.tile([P, P], mybir.dt.bfloat16, tag="wps")
            for _ in range(48):
                nc.tensor.transpose(out=wps[:], in_=ident, identity=ident)

            # global group index per (unit, local group): units in UNIT_KEYS
            # order define both the output-row blocks and each cluster's e-seg
            # columns; EMISSION order below is interleaved (heavy PE groups
            # woven with light ones) to keep the PE stream dense
            gbase_dev = {}
            acc_g = 0
            for u in UNIT_KEYS:
                gbase_dev[u] = acc_g
                acc_g += Ks[u]
            def emit_group(cbase, d, ch, lhsT_of, g):
                osb = workp.tile([P, D_PROJ], mybir.dt.bfloat16, tag="osb")
                pt = pt_sb[cbase]
                ps0 = psump.tile([P, NFREE], mybir.dt.float32, tag="ps")
                ps1 = psump.tile([P, NFREE], mybir.dt.float32, tag="ps")
                pss = [ps0, ps1]
                for kc in range(ch):
                    dk = min(P, d - kc * P)
                    lt = lhsT_of(kc, dk)
                    for oc, ps in enumerate(pss):
                        nc.tensor.matmul(
                            out=ps[:],
                            lhsT=lt,
                            rhs=pt[:dk, kc, oc * NFREE:(oc + 1) * NFREE],
                            start=(kc == 0),
                            stop=(kc == ch - 1),
                        )
                for oc, ps in enumerate(pss):
                    nc.any.tensor_copy(
                        out=osb[:, oc * NFREE:(oc + 1) * NFREE], in_=ps[:]
                    )
                out_eng = nc.sync if g % 2 == 0 else nc.scalar
                out_eng.dma_start(out=out_ext[g * P:(g + 1) * P, :], in_=osb[:])

            # ---- heavy clusters (pre-transposed by the gather) ----
            for j in range(Ks[0]):
                emit_group(
                    0, 1024, 8,
                    lambda kc, dk, _j=j: e0[:dk, _j, kc, :],
                    gbase_dev[0] + j,
                )
            for j in range(Ks[1]):
                emit_group(
                    1, 256, 2,
                    lambda kc, dk, _j=j: e1[:dk, kc, _j * P:(_j + 1) * P],
                    gbase_dev[1] + j,
                )

            # ---- light clusters, phase-batched so the PE stream stays
            # dense: all preprocessing (DVE), then all transposes (PE),
            # then all matmul groups (PE) ----
            # c2: cast f32->bf16 per sub-unit as its gather lands
            ecast = constp.tile([P, max(K2, 1), 64], mybir.dt.bfloat16, tag="ecast")
            for u in [(2, r) for r in range(C2_NSUB)]:
                n = Ks[u]
                if n == 0:
                    continue
                j0 = unit_j0[u]
                nc.vector.tensor_copy(
                    out=ecast[:, j0:j0 + n, :], in_=e2[:, j0:j0 + n, :]
                )
            # c3: one fused mask-multiply (mask shipped from host in aux)
            em_all = constp.tile([P, max(K3, 1), 128], mybir.dt.bfloat16, tag="ema")
            if K3 > 0:
                mask_view = aux_sb[:, 128:128 + K3 * 128].rearrange(
                    "p (j i) -> p j i", j=K3
                )
                nc.vector.tensor_tensor(
                    out=em_all[:], in0=e3[:], in1=mask_view,
                    op=mybir.AluOpType.mult,
                )
            # transposes (PE) + staging copies
            eT2a = constp.tile([64, max(K2, 1) * P], mybir.dt.bfloat16, tag="eT2a")
            for j in range(K2):
                tr2 = psumtr.tile([P, P], mybir.dt.bfloat16, tag="tr")
                nc.tensor.transpose(out=tr2[:64, :], in_=ecast[:, j, :], identity=ident)
                nc.any.tensor_copy(
                    out=eT2a[:, j * P:(j + 1) * P], in_=tr2[:64, :]
                )
            eT3a = constp.tile([P, max(K3, 1) * P], mybir.dt.bfloat16, tag="eT3a")
            for j in range(K3):
                tr3 = psumtr.tile([P, P], mybir.dt.bfloat16, tag="tr")
                nc.tensor.transpose(out=tr3[:], in_=em_all[:, j, :], identity=ident)
                nc.any.tensor_copy(out=eT3a[:, j * P:(j + 1) * P], in_=tr3[:])
            # matmul groups
            for j in range(K2):
                emit_group(
                    2, 64, 1,
                    lambda kc, dk, _j=j: eT2a[:dk, _j * P:(_j + 1) * P],
                    gbase_dev[(2, 0)] + j,
                )
            for j in range(K3):
                emit_group(
                    3, 128, 1,
                    lambda kc, dk, _j=j: eT3a[:dk, _j * P:(_j + 1) * P],
                    gbase_dev[3] + j,
                )

    nc.compile()
    _GRAPH_CACHE[key] = nc
    return nc


def _wrap_idx16(vals, n_slots):
    """int16 values (len <= n_slots, padded with 0) -> [128, n_slots/16] wrapped."""
    full = np.zeros(n_slots, dtype=np.int16)
    full[:len(vals)] = vals
    w = np.zeros((16, n_slots // 16), dtype=np.int16)
    m = np.arange(n_slots)
    w[m % 16, m // 16] = full
    return np.tile(w, (8, 1))


def kernel(inp, emb0, emb1, emb2, emb3, proj0, proj1, proj2, proj3):
    inp = np.asarray(inp)
    embs = [np.asarray(e) for e in (emb0, emb1, emb2, emb3)]
    projs = [np.asarray(p) for p in (proj0, proj1, proj2, proj3)]
    B, S = inp.shape
    flat = inp.reshape(-1).astype(np.int64)
    T = flat.shape[0]

    # ---- host-side bucketing -------------------------------------------
    flat = np.clip(flat, 0, N_TOKEN - 1)
    cluster = np.clip(
        np.searchsorted(np.asarray(CUTOFF_ENDS[1:]), flat, side="right"), 0, 3
    )
    local = flat - np.asarray(CUTOFF_ENDS)[cluster]

    unit_pos = {}
    for u in UNIT_KEYS:
        if u == 0 or u == 1 or u == 3:
            unit_pos[u] = np.nonzero(cluster == u)[0]
        else:
            r = u[1]
            unit_pos[u] = np.nonzero((cluster == 2) & (local // C2_SUB == r))[0]

    core_lists = {u: [unit_pos[u][k::N_CORES] for k in range(N_CORES)]
                  for u in UNIT_KEYS}
    Ks = {
        u: int(-(-max(len(core_lists[u][k]) for k in range(N_CORES)) // P))
        for u in UNIT_KEYS
    }
    G = sum(Ks.values())
    K3 = Ks[3]

    def idxval(u, positions):
        lv = local[positions]
        if u == 0 or u == 1:
            return lv.astype(np.int16)
        if u == 3:
            return (lv // C3_PACK).astype(np.int16)
        return (lv - u[1] * C2_SUB).astype(np.int16)

    NI = 8 * G
    gbase = {}
    acc = 0
    for u in UNIT_KEYS:
        gbase[u] = acc
        acc += Ks[u]

    NAUX = 128 + max(K3, 1) * 128
    blkid = np.arange(128) // 16  # sub-row block of each super-row element

    idx_maps, aux_maps, row_maps = [], [], []
    for k in range(N_CORES):
        cols = []
        row_map = np.full(G * P, -1, dtype=np.int64)
        aux = np.zeros((P, NAUX), dtype=np.float32)
        aux[:, 0:128] = np.eye(P, dtype=np.float32)
        for u in UNIT_KEYS:
            n = Ks[u]
            if n == 0:
                continue
            lst = core_lists[u][k]
            cols.append(_wrap_idx16(idxval(u, lst), n * P))
            m = np.arange(len(lst))
            row_map[(gbase[u] + m // P) * P + (m % P)] = lst
            if u == 3:
                s_arr = np.zeros((P, max(K3, 1)), dtype=np.int64)
                s_arr[m % P, m // P] = local[lst] % C3_PACK
                mask = (blkid[None, None, :] == s_arr[:, :, None])
                aux[:, 128:128 + K3 * 128] = mask.reshape(P, K3 * 128)
        idx_host = (np.concatenate(cols, axis=1) if cols
                    else np.zeros((P, 16), np.int16))
        if idx_host.shape[1] < max(NI, 16):
            pad = np.zeros((P, max(NI, 16) - idx_host.shape[1]), np.int16)
            idx_host = np.concatenate([idx_host, pad], axis=1)
        idx_maps.append(np.ascontiguousarray(idx_host))
        aux_maps.append(aux.astype(BF16))
        row_maps.append(row_map)

    # ---- table/projection prep -----------------------------------------
    emb0b = np.ascontiguousarray(embs[0].astype(BF16))
    emb1b = np.ascontiguousarray(embs[1].astype(BF16))
    emb2f = np.ascontiguousarray(embs[2].astype(np.float32))
    e3flat = embs[3].astype(np.float32)
    pad3 = C3_SROWS * C3_PACK - e3flat.shape[0]
    e3flat = np.concatenate([e3flat, np.zeros((pad3, 16), np.float32)], axis=0)
    emb3p = np.ascontiguousarray(e3flat.reshape(C3_SROWS, 128).astype(BF16))

    pts = {}
    for c, name, pc, ch in ((0, "pt0", 128, 8), (1, "pt1", 128, 2), (2, "pt2", 64, 1)):
        ptc = (projs[c].T.astype(np.float32) * EMB_SCALE).astype(BF16)
        pts[name] = np.ascontiguousarray(
            ptc.reshape(ch, pc, D_PROJ).transpose(1, 0, 2)
        )
    pt3 = projs[3].T.astype(np.float32) * EMB_SCALE
    pts["pt3s"] = np.ascontiguousarray(
        np.tile(pt3, (C3_PACK, 1)).astype(BF16).reshape(128, 1, D_PROJ)
    )

    in_maps = []
    for k in range(N_CORES):
        m = {
            "idx16": idx_maps[k], "aux": aux_maps[k],
            "emb0b": emb0b, "emb1b": emb1b, "emb2f": emb2f, "emb3p": emb3p,
        }
        m.update(pts)
        in_maps.append(m)

    # ---- device --------------------------------------------------------
    nc = _build_graph(Ks)
    res = run_bass_kernel_spmd(
        nc,
        in_maps,
        core_ids=list(range(N_CORES)),
        trace=TRACE,
        trace_cores=TRACE_CORES,
    )
    LAST["res"] = res
    LAST["Ks"] = Ks

    # ---- host-side unshard ---------------------------------------------
    out_full = np.zeros((T, D_PROJ), dtype=np.float32)
    for k in range(N_CORES):
        o = np.asarray(res.results[k]["out"])
        rm = row_maps[k]
        valid = rm >= 0
        out_full[rm[valid]] = o[valid].astype(np.float32)
    return out_full.reshape(B, S, D_PROJ)


# revision 46
# speedup vs baseline: 1.1071x; 1.1071x over previous
"""Adaptive embedding lookup (nn.AdaptiveEmbedding) on 8 TRN2 NeuronCores.

Strategy (data-parallel over tokens, tables replicated, no collectives):

Host:
  - Bucket the 16384 tokens by embedding cluster (4 clusters; cluster 2 is
    further split into 5 sub-ranges of 32000 rows so dma_gather's int16
    indices stay in range, and cluster 3's table is packed 8-rows-per-256B-
    super-row), deal each bucket's tokens round-robin to the 8 cores, pad
    each per-core bucket to a multiple of 128 (one PE tile = one output
    "group" of 128 tokens).
  - Tables/projections pre-converted: emb0/emb1 bf16, emb2 f32 (256B rows
    for dma_gather), emb3 packed bf16 [, 128]; projections pre-transposed,
    pre-scaled by sqrt(d_proj), bf16, chunk-major for direct SBUF DMA.

Device (SPMD, identical graph on all 8 cores, one TileContext):
  - One dma_gather per bucket pulls this core's embedding rows into SBUF
    [128, n_groups, row_elems] (gathered row m lands at partition m%128,
    group m//128); indices are the int16 [128, n/16] wrapped+replicated
    layout the Q7 gather kernel expects.
  - Per 128-token group: (cluster 3: mask-select the 16-elem sub-row inside
    the 128-elem super-row; cluster 2: cast f32->bf16), PE-transpose the
    gathered rows, matmul against projection chunks accumulating in PSUM,
    copy (f32->bf16 cast) to SBUF, DMA the [128, 1024] output rows out.
    The Tile scheduler overlaps gathers, PE work, copies and output DMAs.

Host: inverse-permute the 8 per-core outputs into [8, 2048, 1024] f32.
"""

import numpy as np
import ml_dtypes

import concourse.bacc as bacc
import concourse.bass as bass
import concourse.mybir as mybir
from concourse.bass_utils import run_bass_kernel_spmd
from concourse.tile import TileContext

N_TOKEN = 267735
D_PROJ = 1024
CUTOFF_ENDS = [0, 20000, 40000, 200000, 267735]
D_EMBS = [1024, 256, 64, 16]
EMB_SCALE = float(D_PROJ) ** 0.5
N_CORES = 8
P = 128
NFREE = 512          # psum free-dim per matmul
C2_SUB = 32000       # cluster-2 subtable rows (int16 range)
C2_NSUB = 5
C3_PACK = 8          # cluster-3 rows packed per super-row
C3_SROWS = -(-(CUTOFF_ENDS[4] - CUTOFF_ENDS[3]) // C3_PACK)  # 8467

BF16 = ml_dtypes.bfloat16

# Test-harness knobs (the grader never touches these).
TRACE = False
TRACE_CORES = None
LAST = {}

_GRAPH_CACHE = {}

# unit = gather bucket: 0, 1, (2, r) for sub-range r, 3.
UNIT_KEYS = [0, 1] + [(2, r) for r in range(C2_NSUB)] + [3]


def _build_graph(Ks):
    """Ks: dict unit_key -> group count (0 allowed). Same on all cores."""
    key = tuple(Ks[u] for u in UNIT_KEYS)
    if key in _GRAPH_CACHE:
        return _GRAPH_CACHE[key]

    K3 = Ks[3]
    NI = 8 * sum(Ks.values())          # idx16 columns (8 per group)
    G = sum(Ks.values())               # total output groups
    NAUX = 128 + max(K3, 1) * 128      # [identity | c3 sub-row select mask]
    K2 = sum(Ks[(2, r)] for r in range(C2_NSUB))

    nc = bacc.Bacc("TRN2", debug=False, num_swdge_queues=4)
    idx_ext = nc.declare_dram_parameter("idx16", [P, max(NI, 16)], mybir.dt.int16, False)
    emb0_ext = nc.declare_dram_parameter("emb0b", [20000, 1024], mybir.dt.bfloat16, False)
    emb1_ext = nc.declare_dram_parameter("emb1b", [20000, 256], mybir.dt.bfloat16, False)
    emb2_ext = nc.declare_dram_parameter("emb2f", [160000, 64], mybir.dt.float32, False)
    emb3_ext = nc.declare_dram_parameter("emb3p", [C3_SROWS, 128], mybir.dt.bfloat16, False)
    pt_exts = [
        nc.declare_dram_parameter("pt0", [128, 8, D_PROJ], mybir.dt.bfloat16, False),
        nc.declare_dram_parameter("pt1", [128, 2, D_PROJ], mybir.dt.bfloat16, False),
        nc.declare_dram_parameter("pt2", [64, 1, D_PROJ], mybir.dt.bfloat16, False),
        nc.declare_dram_parameter("pt3s", [128, 1, D_PROJ], mybir.dt.bfloat16, False),
    ]
    aux_ext = nc.declare_dram_parameter("aux", [P, NAUX], mybir.dt.bfloat16, False)
    out_ext = nc.declare_dram_parameter("out", [G * P, D_PROJ], mybir.dt.bfloat16, True)

    with TileContext(nc) as tc:
        with tc.tile_pool(name="const", bufs=1) as constp, \
             tc.tile_pool(name="work", bufs=4) as workp, \
             tc.tile_pool(name="ps_o", bufs=4, space="PSUM") as psump, \
             tc.tile_pool(name="ps_t", bufs=2, space="PSUM") as psumtr, \
             tc.tile_pool(name="ps_w", bufs=1, space="PSUM") as psumw:
            idx_sb = constp.tile([P, max(NI, 16)], mybir.dt.int16, tag="idx")
            nc.sync.dma_start(out=idx_sb[:], in_=idx_ext[:])
            aux_sb = constp.tile([P, NAUX], mybir.dt.bfloat16, tag="aux")
            nc.sync.dma_start(out=aux_sb[:], in_=aux_ext[:])
            ident = aux_sb[:, 0:128]

            # c0/c1 are gathered pre-transposed; c0 is group-major so each
            # 128-token group can be gathered (and consumed) independently
            e0 = constp.tile([P, max(Ks[0], 1), 8, P], mybir.dt.bfloat16, tag="e0")
            e1 = constp.tile([P, 2, max(Ks[1], 1) * P], mybir.dt.bfloat16, tag="e1")
            e2 = constp.tile([P, max(K2, 1), 64], mybir.dt.float32, tag="e2")
            e3 = constp.tile([P, max(K3, 1), 128], mybir.dt.bfloat16, tag="e3")

            # idx16 column offset per unit (indices are packed in UNIT_KEYS order)
            unit_col = {}
            col = 0
            for u in UNIT_KEYS:
                unit_col[u] = col
                col += 8 * Ks[u]
            # group base within each cluster's e-seg, in UNIT_KEYS order
            unit_j0 = {}
            jb = {0: 0, 1: 0, 2: 0, 3: 0}
            for u in UNIT_KEYS:
                cbase = 2 if isinstance(u, tuple) else u
                unit_j0[u] = jb[cbase]
                jb[cbase] += Ks[u]

            # gather emission order: c0 split per group across queues 0/1 so
            # the first matmuls can start as soon as group 0 lands; c1/c3
            # early on the other queues; cluster-2 sub-gathers fill in
            gather_list = []  # (unit, group_lo, n_groups)
            for glo in range(Ks[0]):
                gather_list.append((0, glo, 1))
            for u in [1, 3, (2, 0), (2, 1), (2, 2), (2, 3), (2, 4)]:
                if Ks[u] > 0:
                    gather_list.append((u, 0, Ks[u]))
            for qi, (u, glo, n) in enumerate(gather_list):
                j0 = unit_j0[u] + glo
                tr_mode = u in (0, 1)
                if u == 0:
                    dst, tab, elem = e0[:, glo, :, :], emb0_ext[:], 1024
                elif u == 1:
                    dst, tab, elem = e1[:], emb1_ext[:], 256
                elif u == 3:
                    dst, tab, elem = e3[:, j0:j0 + n, :], emb3_ext[:], 128
                else:
                    r = u[1]
                    dst, tab, elem = (
                        e2[:, j0:j0 + n, :],
                        emb2_ext[r * C2_SUB:(r + 1) * C2_SUB, :],
                        64,
                    )
                c0 = unit_col[u] + 8 * glo
                nc.gpsimd.dma_gather(
                    dst, tab, idx_sb[:, c0:c0 + 8 * n], n * P, n * P, elem,
                    transpose=tr_mode,
                    queue_num=qi % 4,
                )

            pt_sb = []
            for i, (pc, ch) in enumerate(((128, 8), (128, 2), (64, 1), (128, 1))):
                t = constp.tile([pc, ch, D_PROJ], mybir.dt.bfloat16, tag=f"pt{i}")
                nc.sync.dma_start(out=t[:], in_=pt_exts[i][:])
                pt_sb.append(t)

            # PE warmup: a dense burst of throwaway transposes keeps the PE
            # clock ramping toward HAM while the gathers are still in flight
            wps = psumw.tile([P, P], mybir.dt.bfloat16, tag="wps")
            for _ in range(48):
                nc.tensor.transpose(out=wps[:], in_=ident, identity=ident)

            # global group index per (unit, local group): units in UNIT_KEYS
            # order define both the output-row blocks and each cluster's e-seg
            # columns; EMISSION order below is interleaved (heavy PE groups
            # woven with light ones) to keep the PE stream dense
            gbase_dev = {}
            acc_g = 0
            for u in UNIT_KEYS:
                gbase_dev[u] = acc_g
                acc_g += Ks[u]
            last_pe_inst = [None]

            def emit_group(cbase, d, ch, lhsT_of, g):
                osb = workp.tile([P, D_PROJ], mybir.dt.bfloat16, tag="osb")
                pt = pt_sb[cbase]
                ps0 = psump.tile([P, NFREE], mybir.dt.float32, tag="ps")
                ps1 = psump.tile([P, NFREE], mybir.dt.float32, tag="ps")
                pss = [ps0, ps1]
                for kc in range(ch):
                    dk = min(P, d - kc * P)
                    lt = lhsT_of(kc, dk)
                    for oc, ps in enumerate(pss):
                        mm = nc.tensor.matmul(
                            out=ps[:],
                            lhsT=lt,
                            rhs=pt[:dk, kc, oc * NFREE:(oc + 1) * NFREE],
                            start=(kc == 0),
                            stop=(kc == ch - 1),
                        )
                        last_pe_inst[0] = mm.ins
                for oc, ps in enumerate(pss):
                    nc.any.tensor_copy(
                        out=osb[:, oc * NFREE:(oc + 1) * NFREE], in_=ps[:]
                    )
                out_eng = nc.sync if g % 2 == 0 else nc.scalar
                out_eng.dma_start(out=out_ext[g * P:(g + 1) * P, :], in_=osb[:])

            # ---- heavy clusters (pre-transposed by the gather) ----
            for j in range(Ks[0]):
                emit_group(
                    0, 1024, 8,
                    lambda kc, dk, _j=j: e0[:dk, _j, kc, :],
                    gbase_dev[0] + j,
                )
            for j in range(Ks[1]):
                emit_group(
                    1, 256, 2,
                    lambda kc, dk, _j=j: e1[:dk, kc, _j * P:(_j + 1) * P],
                    gbase_dev[1] + j,
                )

            # ---- light clusters, phase-batched so the PE stream stays
            # dense: all preprocessing (DVE), then all transposes (PE),
            # then all matmul groups (PE) ----
            # c2: cast f32->bf16 per sub-unit as its gather lands
            ecast = constp.tile([P, max(K2, 1), 64], mybir.dt.bfloat16, tag="ecast")
            for u in [(2, r) for r in range(C2_NSUB)]:
                n = Ks[u]
                if n == 0:
                    continue
                j0 = unit_j0[u]
                nc.vector.tensor_copy(
                    out=ecast[:, j0:j0 + n, :], in_=e2[:, j0:j0 + n, :]
                )
            # c3: one fused mask-multiply (mask shipped from host in aux)
            em_all = constp.tile([P, max(K3, 1), 128], mybir.dt.bfloat16, tag="ema")
            if K3 > 0:
                mask_view = aux_sb[:, 128:128 + K3 * 128].rearrange(
                    "p (j i) -> p j i", j=K3
                )
                nc.vector.tensor_tensor(
                    out=em_all[:], in0=e3[:], in1=mask_view,
                    op=mybir.AluOpType.mult,
                )
            # transposes (PE) + staging copies.  PE's stream is in-order and
            # the scheduler may otherwise hoist these (whose data lands LAST)
            # ahead of the heavy matmuls (whose data lands FIRST) — pin them
            # behind the heavy section with no-sync scheduling edges.
            import bass_rust as _br
            heavy_tail = last_pe_inst[0]
            eT2a = constp.tile([64, max(K2, 1) * P], mybir.dt.bfloat16, tag="eT2a")
            for j in range(K2):
                tr2 = psumtr.tile([P, P], mybir.dt.bfloat16, tag="tr")
                ti = nc.tensor.transpose(
                    out=tr2[:64, :], in_=ecast[:, j, :], identity=ident
                )
                if heavy_tail is not None:
                    _br.add_dep_helper(
                        ti.ins, heavy_tail, sync=False,
                        reason="light transposes after heavy matmuls",
                    )
                nc.any.tensor_copy(
                    out=eT2a[:, j * P:(j + 1) * P], in_=tr2[:64, :]
                )
            eT3a = constp.tile([P, max(K3, 1) * P], mybir.dt.bfloat16, tag="eT3a")
            for j in range(K3):
                tr3 = psumtr.tile([P, P], mybir.dt.bfloat16, tag="tr")
                ti = nc.tensor.transpose(
                    out=tr3[:], in_=em_all[:, j, :], identity=ident
                )
                if heavy_tail is not None:
                    _br.add_dep_helper(
                        ti.ins, heavy_tail, sync=False,
                        reason="light transposes after heavy matmuls",
                    )
                nc.any.tensor_copy(out=eT3a[:, j * P:(j + 1) * P], in_=tr3[:])
            # matmul groups
            for j in range(K2):
                emit_group(
                    2, 64, 1,
                    lambda kc, dk, _j=j: eT2a[:dk, _j * P:(_j + 1) * P],
                    gbase_dev[(2, 0)] + j,
                )
            for j in range(K3):
                emit_group(
                    3, 128, 1,
                    lambda kc, dk, _j=j: eT3a[:dk, _j * P:(_j + 1) * P],
                    gbase_dev[3] + j,
                )

    nc.compile()
    _GRAPH_CACHE[key] = nc
    return nc


def _wrap_idx16(vals, n_slots):
    """int16 values (len <= n_slots, padded with 0) -> [128, n_slots/16] wrapped."""
    full = np.zeros(n_slots, dtype=np.int16)
    full[:len(vals)] = vals
    w = np.zeros((16, n_slots // 16), dtype=np.int16)
    m = np.arange(n_slots)
    w[m % 16, m // 16] = full
    return np.tile(w, (8, 1))


def kernel(inp, emb0, emb1, emb2, emb3, proj0, proj1, proj2, proj3):
    inp = np.asarray(inp)
    embs = [np.asarray(e) for e in (emb0, emb1, emb2, emb3)]
    projs = [np.asarray(p) for p in (proj0, proj1, proj2, proj3)]
    B, S = inp.shape
    flat = inp.reshape(-1).astype(np.int64)
    T = flat.shape[0]

    # ---- host-side bucketing -------------------------------------------
    flat = np.clip(flat, 0, N_TOKEN - 1)
    cluster = np.clip(
        np.searchsorted(np.asarray(CUTOFF_ENDS[1:]), flat, side="right"), 0, 3
    )
    local = flat - np.asarray(CUTOFF_ENDS)[cluster]

    unit_pos = {}
    for u in UNIT_KEYS:
        if u == 0 or u == 1 or u == 3:
            unit_pos[u] = np.nonzero(cluster == u)[0]
        else:
            r = u[1]
            unit_pos[u] = np.nonzero((cluster == 2) & (local // C2_SUB == r))[0]

    core_lists = {u: [unit_pos[u][k::N_CORES] for k in range(N_CORES)]
                  for u in UNIT_KEYS}
    Ks = {
        u: int(-(-max(len(core_lists[u][k]) for k in range(N_CORES)) // P))
        for u in UNIT_KEYS
    }
    G = sum(Ks.values())
    K3 = Ks[3]

    def idxval(u, positions):
        lv = local[positions]
        if u == 0 or u == 1:
            return lv.astype(np.int16)
        if u == 3:
            return (lv // C3_PACK).astype(np.int16)
        return (lv - u[1] * C2_SUB).astype(np.int16)

    NI = 8 * G
    gbase = {}
    acc = 0
    for u in UNIT_KEYS:
        gbase[u] = acc
        acc += Ks[u]

    NAUX = 128 + max(K3, 1) * 128
    blkid = np.arange(128) // 16  # sub-row block of each super-row element

    idx_maps, aux_maps, row_maps = [], [], []
    for k in range(N_CORES):
        cols = []
        row_map = np.full(G * P, -1, dtype=np.int64)
        aux = np.zeros((P, NAUX), dtype=np.float32)
        aux[:, 0:128] = np.eye(P, dtype=np.float32)
        for u in UNIT_KEYS:
            n = Ks[u]
            if n == 0:
                continue
            lst = core_lists[u][k]
            cols.append(_wrap_idx16(idxval(u, lst), n * P))
            m = np.arange(len(lst))
            row_map[(gbase[u] + m // P) * P + (m % P)] = lst
            if u == 3:
                s_arr = np.zeros((P, max(K3, 1)), dtype=np.int64)
                s_arr[m % P, m // P] = local[lst] % C3_PACK
                mask = (blkid[None, None, :] == s_arr[:, :, None])
                aux[:, 128:128 + K3 * 128] = mask.reshape(P, K3 * 128)
        idx_host = (np.concatenate(cols, axis=1) if cols
                    else np.zeros((P, 16), np.int16))
        if idx_host.shape[1] < max(NI, 16):
            pad = np.zeros((P, max(NI, 16) - idx_host.shape[1]), np.int16)
            idx_host = np.concatenate([idx_host, pad], axis=1)
        idx_maps.append(np.ascontiguousarray(idx_host))
        aux_maps.append(aux.astype(BF16))
        row_maps.append(row_map)

    # ---- table/projection prep -----------------------------------------
    emb0b = np.ascontiguousarray(embs[0].astype(BF16))
    emb1b = np.ascontiguousarray(embs[1].astype(BF16))
    emb2f = np.ascontiguousarray(embs[2].astype(np.float32))
    e3flat = embs[3].astype(np.float32)
    pad3 = C3_SROWS * C3_PACK - e3flat.shape[0]
    e3flat = np.concatenate([e3flat, np.zeros((pad3, 16), np.float32)], axis=0)
    emb3p = np.ascontiguousarray(e3flat.reshape(C3_SROWS, 128).astype(BF16))

    pts = {}
    for c, name, pc, ch in ((0, "pt0", 128, 8), (1, "pt1", 128, 2), (2, "pt2", 64, 1)):
        ptc = (projs[c].T.astype(np.float32) * EMB_SCALE).astype(BF16)
        pts[name] = np.ascontiguousarray(
            ptc.reshape(ch, pc, D_PROJ).transpose(1, 0, 2)
        )
    pt3 = projs[3].T.astype(np.float32) * EMB_SCALE
    pts["pt3s"] = np.ascontiguousarray(
        np.tile(pt3, (C3_PACK, 1)).astype(BF16).reshape(128, 1, D_PROJ)
    )

    in_maps = []
    for k in range(N_CORES):
        m = {
            "idx16": idx_maps[k], "aux": aux_maps[k],
            "emb0b": emb0b, "emb1b": emb1b, "emb2f": emb2f, "emb3p": emb3p,
        }
        m.update(pts)
        in_maps.append(m)

    # ---- device --------------------------------------------------------
    nc = _build_graph(Ks)
    res = run_bass_kernel_spmd(
        nc,
        in_maps,
        core_ids=list(range(N_CORES)),
        trace=TRACE,
        trace_cores=TRACE_CORES,
    )
    LAST["res"] = res
    LAST["Ks"] = Ks

    # ---- host-side unshard ---------------------------------------------
    out_full = np.zeros((T, D_PROJ), dtype=np.float32)
    for k in range(N_CORES):
        o = np.asarray(res.results[k]["out"])
        rm = row_maps[k]
        valid = rm >= 0
        out_full[rm[valid]] = o[valid].astype(np.float32)
    return out_full.reshape(B, S, D_PROJ)
